# revision 34
# baseline (speedup 1.0000x reference)
"""MultiHeadAttention (B=4, N=2048, E=1024, H=16) on 8 TRN2 NeuronCores.

Sharding: core c handles batch b = c//2 and head-half hh = c%2 (8 heads,
512 embed dims). Each core computes Q/K/V projections for its 8 heads,
attention, and a partial output projection (contraction over its 512 c-dims).
Host sums the two partials per batch and adds the output bias.

Design: the kernel is scheduled to be ACT(exp)-bound. Per core the exp
work is 32 x [2048 x 512] = 33.5M elements = ~293us on the scalar engine
(1 elem/lane/cycle @ 1.2 GHz + 352-cycle per-ACTIVATE overhead); every
other engine is kept under that pace:

 - Attention runs in 16 head-PAIR slots (qb, t): heads (2t, 2t+1) live at
   partitions 0:64 / 64:128 of QT[t]/KT[t], so the two per-chunk score
   matmuls (contraction d=64) land on disjoint PE row groups
   (tile_position (0,0) / (64,0)) and execute CONCURRENTLY -- halving
   score time vs. serial d=64 matmuls.
 - Per chunk (1 key tile x 2 heads): 2 concurrent score MMs -> st PSUM
   [128,2,512], one exp ACTIVATE [128, 1024] -> at bf16, 2 attn@V MMs
   (V stationary [128,65] with a ones column producing the softmax
   denominator in row 64).
 - Scores are issued 2 chunks ahead of exp (st double-buffered); attnv
   lags exp by 2 chunks; the PE never sits between ACT and itself on the
   critical path, so ACT runs back-to-back.
 - Softmax normalization is moved OFF the critical path: ot PSUM banks
   are drained to SBUF immediately (freeing them for the next pair =
   single-buffered ot), denominators of both heads are DMA'd into rows
   0/32 of one tile so ONE DVE reciprocal (iterative, ~6.5 cyc/elem)
   serves the pair, the reciprocal row is broadcast to 64 partitions by
   two col-tiled K=1 matmuls into one PSUM bank, and DVE multiplies
   produce otn [128, 512] bf16 (head B's value half is partition-shifted
   64->128 via an SBUF->SBUF DMA since DVE lanes cannot cross partitions).
 - Projections: K fully + Q(qb0,t0) upfront (scores start ~15us in);
   V projection, remaining Q projection, output projection and the
   reciprocal-broadcast matmuls are issued as PE fillers inside the
   ACT-bound slots.
 - PSUM budget (8 banks): st 2x[128,2,512] (4) + ot_A + ot_B (2) +
   shared proj/bcast/outproj pool (2).
"""
import sys

sys.path.insert(0, "/opt/trn_rl_repo")

import numpy as np
import ml_dtypes

B, N, E = 4, 2048, 1024
NCORES = 8
HH = 512          # embed dims (8 heads x 64) per core
D = 64
NHEAD = 8         # heads per core
NKT = 16          # 128-wide key tiles
NPAIR = 16        # head-pair slots: pair p = (qb, t) = (p//4, p%4)

_cache = {}


def _build():
    import concourse.mybir as mybir
    import concourse.tile as tile
    import concourse.bacc as bacc

    F32 = mybir.dt.float32
    BF16 = mybir.dt.bfloat16
    EXP = mybir.ActivationFunctionType.Exp

    nc = bacc.Bacc(trn_type="TRN2")

    xtq = nc.dram_tensor("xtq", [E, N], BF16, kind="ExternalInput")
    xtk = nc.dram_tensor("xtk", [E, N], BF16, kind="ExternalInput")
    xtv = nc.dram_tensor("xtv", [E, N], BF16, kind="ExternalInput")
    wqt = nc.dram_tensor("wqt", [E, HH], BF16, kind="ExternalInput")
    wkt = nc.dram_tensor("wkt", [E, HH], BF16, kind="ExternalInput")
    wvt = nc.dram_tensor("wvt", [E, HH], BF16, kind="ExternalInput")
    wot = nc.dram_tensor("wot", [HH, E], BF16, kind="ExternalInput")
    bq = nc.dram_tensor("bq", [HH], F32, kind="ExternalInput")
    bk = nc.dram_tensor("bk", [HH], F32, kind="ExternalInput")
    bv = nc.dram_tensor("bv", [HH], F32, kind="ExternalInput")
    po = nc.dram_tensor("po", [E, N], F32, kind="ExternalOutput")

    with tile.TileContext(nc) as tc:
        with (
            tc.tile_pool(name="consts", bufs=1) as consts,
            tc.tile_pool(name="qk", bufs=1) as qk_pool,
            tc.tile_pool(name="vx", bufs=1) as v_pool,
            tc.tile_pool(name="wop", bufs=1) as wo_pool,
            tc.tile_pool(name="wp", bufs=1) as w_pool,
            tc.tile_pool(name="xt", bufs=5) as xt_pool,
            tc.tile_pool(name="at", bufs=17) as at_pool,
            tc.tile_pool(name="osb", bufs=2) as osb_pool,
            tc.tile_pool(name="dn", bufs=2) as dn_pool,
            tc.tile_pool(name="otn", bufs=2) as otn_pool,
            tc.tile_pool(name="ob", bufs=2) as ob_pool,
            tc.tile_pool(name="ojs", bufs=2) as ojs_pool,
            tc.tile_pool(name="st", bufs=2, space="PSUM") as st_ps,
            tc.tile_pool(name="ot", bufs=1, space="PSUM") as ot_ps,
            tc.tile_pool(name="pb", bufs=2, space="PSUM") as pb_ps,
        ):
            # ---------------- constants ----------------
            ones_b = consts.tile([128, 128], BF16)
            nc.vector.memset(ones_b, 1.0)
            onescol_f = consts.tile([128, NHEAD, 1], F32)
            nc.vector.memset(onescol_f, 1.0)
            # preload the ACT exp table during the ramp (~2.7us
            # ACT_TABLE_LOAD otherwise lands on the first attention exp)
            exp_warm = consts.tile([1, 8], F32)
            nc.vector.memset(exp_warm, 0.0)
            exp_warm_o = consts.tile([1, 8], F32)
            nc.scalar.activation(exp_warm_o, exp_warm, EXP)

            bq_t = consts.tile([128, 4], F32)
            bk_t = consts.tile([128, 4], F32)
            nc.sync.dma_start(out=bq_t, in_=bq.ap().rearrange("(t p) -> p t", p=128))
            nc.sync.dma_start(out=bk_t, in_=bk.ap().rearrange("(t p) -> p t", p=128))
            bv_row = consts.tile([1, HH], F32)
            nc.sync.dma_start(out=bv_row, in_=bv.ap().rearrange("(a n) -> a n", a=1))
            bv_row_b = consts.tile([1, HH], BF16)
            nc.vector.tensor_copy(bv_row_b, bv_row)
            bv_bc = consts.tile([128, HH], F32)

            # persistent activations
            QT = [qk_pool.tile([128, N], BF16, tag=f"qt{t}", name=f"qt{t}") for t in range(4)]
            KT = [qk_pool.tile([128, N], BF16, tag=f"kt{t}", name=f"kt{t}") for t in range(4)]
            VE = [v_pool.tile([128, NHEAD, D + 1], BF16, tag=f"ve{g}", name=f"ve{g}") for g in range(16)]
            wo_t = wo_pool.tile([128, 4, E], BF16, tag="wo")

            # ---------------- weight / x loads ----------------
            wts = {}

            def load_w(nm, wdram, engine=None):
                wt = w_pool.tile([128, 8, HH], BF16, tag=f"w{nm}", name=f"wt_{nm}")
                (engine or nc.sync).dma_start(
                    out=wt, in_=wdram.ap().rearrange("(kt p) n -> p kt n", p=128)
                )
                wts[nm] = wt

            xts = {}

            def load_x(which, xdram, th, engine=None):
                # 'q' tiles live across several slots (QP(qb, dt) spread over
                # 3 slots) -- separate 2-buf tag so k/v rotation can't evict
                # them before their late readers are emitted
                tag, bufs = ("xq", 2) if which == "q" else ("xt", None)
                xt = xt_pool.tile(
                    [128, 8, 512], BF16, tag=tag, bufs=bufs, name=f"x{which}{th}"
                )
                (engine or nc.sync).dma_start(
                    out=xt,
                    in_=xdram.ap().rearrange("(kt p) n -> p kt n", p=128)[
                        :, :, 512 * th : 512 * (th + 1)
                    ],
                )
                xts[(which, th)] = xt

            def qk_group(which, th, dt):
                xt = xts[(which, th)]
                wt = wts[which]
                dest = (QT if which == "q" else KT)[dt]
                bias = bq_t if which == "q" else bk_t
                ps = pb_ps.tile([128, 512], F32, tag="pb", name=f"pp{which}{th}{dt}")
                for kt in range(8):
                    nc.tensor.matmul(
                        ps,
                        wt[:, kt, 128 * dt : 128 * (dt + 1)],
                        xt[:, kt, :],
                        start=(kt == 0),
                        stop=(kt == 7),
                    )
                nc.vector.tensor_scalar_add(
                    dest[:, 512 * th : 512 * (th + 1)], ps, bias[:, dt : dt + 1]
                )

            vp_emitted = [False] * 16

            def v_group(th, tt):
                xt = xts[("v", th)]
                g = 4 * th + tt
                ps = pb_ps.tile([128, 512], F32, tag="pb", name=f"pv{g}")
                for kt in range(8):
                    nc.tensor.matmul(
                        ps,
                        xt[:, kt, 128 * tt : 128 * (tt + 1)],
                        wts["v"][:, kt, :],
                        start=(kt == 0),
                        stop=(kt == 7),
                    )
                nc.vector.tensor_add(
                    VE[g][:, :, 0:D],
                    ps.rearrange("p (h d) -> p h d", h=NHEAD),
                    bv_bc.rearrange("p (h d) -> p h d", h=NHEAD),
                )
                nc.vector.tensor_copy(VE[g][:, :, D : D + 1], onescol_f)
                vp_emitted[g] = True

            def load_wo():
                nc.gpsimd.dma_start(
                    out=wo_t, in_=wot.ap().rearrange("(ct p) n -> p ct n", p=128)
                )

            # ---------------- attention primitives ----------------
            at_tiles = {}
            ot_tiles = {}
            osb_tiles = {}
            r2_tiles = {}
            otn_tiles = {}
            bc_queue = []
            exp_count = [0]   # chunks exp'd (global)
            av_count = [0]    # chunks attnv'd (global)

            def sc(p, c):
                qb, t = divmod(p, 4)
                q0 = 512 * qb
                st = st_ps.tile([128, 2, 512], F32, tag="st", name=f"st{p}_{c}")
                for i, par in enumerate((0, 64)):
                    nc.tensor.matmul(
                        st[:, i, :],
                        KT[t][par : par + 64, 128 * c : 128 * (c + 1)],
                        QT[t][par : par + 64, q0 : q0 + 512],
                        start=True,
                        stop=True,
                    )
                at = at_pool.tile([128, 2, 512], BF16, tag="at", name=f"at{p}_{c}")
                nc.scalar.activation(at, st, EXP, scale=0.125)
                at_tiles[(p, c)] = at
                exp_count[0] += 1

            def attnv_one(p, c):
                qb, t = divmod(p, 4)
                if c == 0:
                    ot_tiles[(p, 0)] = ot_ps.tile(
                        [128, 512], F32, tag="ota", name=f"ota{p}"
                    )
                    ot_tiles[(p, 1)] = ot_ps.tile(
                        [128, 512], F32, tag="otb", name=f"otb{p}"
                    )
                at = at_tiles.pop((p, c))
                for i in (0, 1):
                    h = 2 * t + i
                    nc.tensor.matmul(
                        ot_tiles[(p, i)][0:65, :],
                        VE[c][:, h, :],
                        at[:, i, :],
                        start=(c == 0),
                        stop=(c == 15),
                    )
                av_count[0] += 1
                if c == 15:
                    norm_copies(p)

            def norm_copies(p):
                # denominator rows first (tiny copies) so the den DMAs and
                # the reciprocal start ~1.5us earlier -- keeps the bc
                # broadcast matmul at c==6 of the next slot from stalling PE
                ota, otb = ot_tiles.pop((p, 0)), ot_tiles.pop((p, 1))
                rowa = dn_pool.tile([1, 512], F32, tag="rowa", bufs=1, name=f"rwa{p}")
                rowb = dn_pool.tile([1, 512], F32, tag="rowb", bufs=1, name=f"rwb{p}")
                nc.vector.tensor_copy(rowa, ota[64:65, :])
                nc.vector.tensor_copy(rowb, otb[64:65, :])
                den = dn_pool.tile([33, 512], F32, tag="den", bufs=1, name=f"den{p}")
                nc.sync.dma_start(out=den[0:1, :], in_=rowa)
                nc.sync.dma_start(out=den[32:33, :], in_=rowb)
                # drain ot value rows to SBUF (frees PSUM for the next pair)
                oa = osb_pool.tile([64, 512], F32, tag="osba", name=f"osba{p}")
                ob_ = osb_pool.tile([64, 512], F32, tag="osbb", name=f"osbb{p}")
                nc.vector.tensor_copy(oa, ota[0:64, :])
                nc.vector.tensor_copy(ob_, otb[0:64, :])
                r2 = dn_pool.tile([33, 512], BF16, tag="r2", bufs=1, name=f"r2{p}")
                with nc.allow_low_precision(reason="bf16 softmax recip"):
                    nc.vector.reciprocal(r2, den)
                # head B's value half must live at partitions 64:128 for the
                # out-projection moving operand; DVE lanes can't cross
                # partitions, so shift via SBUF->SBUF DMA
                obig = ob_pool.tile([128, 512], F32, tag="ob", name=f"ob{p}")
                nc.sync.dma_start(out=obig[64:128, :], in_=ob_)
                osb_tiles[p] = (oa, obig)
                r2_tiles[p] = r2
                bc_queue.append(p)

            oj_queue = []

            def bc_mul(p):
                qb, t = divmod(p, 4)
                r2 = r2_tiles.pop(p)
                bc = pb_ps.tile([128, 512], F32, tag="pb", name=f"bc{p}")
                nc.tensor.matmul(
                    bc[0:64, :], ones_b[0:1, 0:64], r2[0:1, :], start=True, stop=True
                )
                nc.tensor.matmul(
                    bc[64:128, :], ones_b[32:33, 0:64], r2[32:33, :],
                    start=True, stop=True,
                )
                oa, obig = osb_tiles.pop(p)
                otn = otn_pool.tile([128, 512], BF16, tag=f"otn{t}", name=f"otn{p}")
                nc.vector.tensor_mul(otn[0:64, :], oa[0:64, :], bc[0:64, :])
                nc.vector.tensor_mul(otn[64:128, :], obig[64:128, :], bc[64:128, :])
                otn_tiles[(qb, t)] = otn
                if t == 3:
                    oj_queue.extend((qb, jt) for jt in range(8))

            def oj_group(qb, jt, pool_tag=None):
                pool, tag = pool_tag or (pb_ps, "pb")
                pj = pool.tile([128, 512], F32, tag=tag, name=f"oj{qb}_{jt}")
                for ct in range(4):
                    nc.tensor.matmul(
                        pj,
                        wo_t[:, ct, 128 * jt : 128 * (jt + 1)],
                        otn_tiles[(qb, ct)],
                        start=(ct == 0),
                        stop=(ct == 3),
                    )
                ojsb = ojs_pool.tile([128, 512], F32, tag="ojs", name=f"ojsb{qb}_{jt}")
                nc.vector.tensor_copy(ojsb, pj)
                nc.sync.dma_start(
                    out=po.ap()[128 * jt : 128 * (jt + 1), 512 * qb : 512 * (qb + 1)],
                    in_=ojsb,
                )

            # attnv drain: lag >= min_lag chunks behind exp (keeps the PE
            # from stalling on ACT mid-slot); gated on VE emission
            def drain_attnv(max_items, min_lag=2):
                n = 0
                while n < max_items and av_count[0] <= exp_count[0] - min_lag:
                    p_a, c_a = divmod(av_count[0], 16)
                    if not vp_emitted[c_a]:
                        break
                    attnv_one(p_a, c_a)
                    n += 1

            # ---------------- ramp DMAs ----------------
            # xtv loads are emitted just-in-time inside the slot-0 script so
            # their xt-pool buffer reuse lands after the K-proj readers of
            # the evicted xtk tiles are emitted (pool WAR only orders
            # against already-emitted readers).
            # spread across the three DMA-capable queues: K path on sync,
            # Q path on the scalar queue (ACT is idle until the first exp),
            # V path + wo on the gpsimd software DGE
            load_w("k", wkt)
            load_x("k", xtk, 0)
            load_w("q", wqt, engine=nc.scalar)
            load_x("q", xtq, 0, engine=nc.scalar)
            for th in (1, 2, 3):
                load_x("k", xtk, th)
            load_w("v", wvt, engine=nc.gpsimd)

            # bv broadcast to all partitions via K=1 matmul
            bc0 = pb_ps.tile([128, HH], F32, tag="pb", name="bvbc")
            nc.tensor.matmul(bc0, ones_b[0:1, :], bv_row_b, start=True, stop=True)
            nc.vector.tensor_copy(bv_bc, bc0)

            # ---------------- slot 0 (pair 0): ramp script ----------------
            # Pair 0 = (qb0, t0): chunk c needs only KP(th=c//4, dt=0), so
            # KP(th, 0) goes on the critical path and the dt>0 K-proj groups
            # (KT[1..3], needed from slot 1 on) interleave behind it.
            qk_group("k", 0, 0)
            qk_group("q", 0, 0)
            load_x("v", xtv, 0, engine=nc.gpsimd)   # 5th xt buf (fresh)
            sc(0, 0)
            sc(0, 1)
            qk_group("k", 1, 0)
            qk_group("k", 0, 1)
            sc(0, 2)
            sc(0, 3)
            qk_group("k", 2, 0)
            qk_group("k", 0, 2)
            sc(0, 4)
            sc(0, 5)
            qk_group("k", 3, 0)
            qk_group("k", 0, 3)
            load_x("v", xtv, 1, engine=nc.gpsimd)   # reuses xtk0 buf (KP(0,*) emitted)
            sc(0, 6)
            sc(0, 7)
            qk_group("k", 1, 1)
            qk_group("k", 2, 1)
            sc(0, 8)
            sc(0, 9)
            qk_group("k", 3, 1)
            qk_group("k", 1, 2)
            sc(0, 10)
            sc(0, 11)
            qk_group("k", 2, 2)
            qk_group("k", 3, 2)
            sc(0, 12)
            sc(0, 13)
            qk_group("k", 1, 3)
            load_x("v", xtv, 2, engine=nc.gpsimd)   # reuses xtk1 buf (KP(1,*) emitted)
            sc(0, 14)
            sc(0, 15)
            qk_group("k", 2, 3)
            qk_group("k", 3, 3)
            load_x("v", xtv, 3, engine=nc.gpsimd)   # reuses xtk2 buf (KP(2,*) emitted)
            qk_group("q", 0, 1)
            # V-proj for the first 10 key tiles while ACT drains the pair-0
            # exp backlog (PE is otherwise idle at slot-0 end)
            v_group(0, 0)
            v_group(0, 1)
            drain_attnv(2)
            v_group(0, 2)
            v_group(0, 3)
            drain_attnv(2)
            v_group(1, 0)
            v_group(1, 1)
            drain_attnv(2)
            v_group(1, 2)
            v_group(1, 3)
            drain_attnv(2)
            v_group(2, 0)
            v_group(2, 1)
            load_wo()
            load_x("q", xtq, 1)
            drain_attnv(4)

            # ---------------- slots 1..15 ----------------
            # filler queues per slot (projection groups during the ramp,
            # Q-projection blocks just-in-time for later q-blocks).
            # All V-proj groups go in slot 1 so attnv (VE-gated) never
            # lags exp by more than the at-pool depth.
            fillers = {
                1: [lambda: v_group(2, 2), lambda: v_group(2, 3),
                    lambda: v_group(3, 0), lambda: v_group(3, 1),
                    lambda: v_group(3, 2), lambda: v_group(3, 3),
                    lambda: qk_group("q", 0, 2)],
                2: [lambda: qk_group("q", 0, 3)],
                3: [lambda: qk_group("q", 1, 0)],
                4: [lambda: qk_group("q", 1, 1)],
                5: [lambda: qk_group("q", 1, 2), lambda: load_x("q", xtq, 2)],
                6: [lambda: qk_group("q", 1, 3)],
                7: [lambda: qk_group("q", 2, 0)],
                8: [lambda: qk_group("q", 2, 1)],
                9: [lambda: qk_group("q", 2, 2), lambda: load_x("q", xtq, 3)],
                10: [lambda: qk_group("q", 2, 3)],
                11: [lambda: qk_group("q", 3, 0)],
                12: [lambda: qk_group("q", 3, 1)],
                13: [lambda: qk_group("q", 3, 2)],
                14: [lambda: qk_group("q", 3, 3)],
            }
            for s in range(1, NPAIR):
                fq = list(fillers.get(s, []))
                filler_slots = set(range(16)) if s == 1 else {2, 5}
                for c in range(16):
                    sc(s, c)
                    if fq and c in filler_slots:
                        fq.pop(0)()
                    if c in (6, 10) and bc_queue:
                        bc_mul(bc_queue.pop(0))
                    if c in (8, 12) and oj_queue:
                        oj_group(*oj_queue.pop(0))
                    drain_attnv(3)
                while fq:
                    fq.pop(0)()
                # min_lag=1: emit the slot's last two attnv chunks (and the
                # norm chain) at the slot boundary so the reciprocal is done
                # before the bc matmul at c==6 of the next slot
                drain_attnv(6, min_lag=1)

            # ---------------- tail ----------------
            while av_count[0] < NPAIR * 16:
                drain_attnv(16, min_lag=1)
            while bc_queue:
                bc_mul(bc_queue.pop(0))
            # tail out-projections 4-wide: the attnv accumulator banks are
            # dead after pair 15's drain, reuse them as extra oj slots
            tail_pools = [(pb_ps, "pb"), (ot_ps, "ota"), (pb_ps, "pb"), (ot_ps, "otb")]
            i = 0
            while oj_queue:
                oj_group(*oj_queue.pop(0), pool_tag=tail_pools[i % 4])
                i += 1

    nc.compile()
    return nc


def _get_nc():
    if "nc" not in _cache:
        _cache["nc"] = _build()
    return _cache["nc"]


def kernel(query, key, value, Wq, bq, Wk, bk, Wv, bv, Wo, bo):
    from concourse.bass_utils import run_bass_kernel_spmd

    nc = _get_nc()

    query = np.asarray(query, dtype=np.float32)
    key = np.asarray(key, dtype=np.float32)
    value = np.asarray(value, dtype=np.float32)
    Wq, Wk, Wv, Wo = (np.asarray(w, dtype=np.float32) for w in (Wq, Wk, Wv, Wo))
    bq, bk, bv, bo = (np.asarray(b, dtype=np.float32) for b in (bq, bk, bv, bo))

    in_maps = []
    for c in range(NCORES):
        b, hh = c // 2, c % 2
        cols = slice(HH * hh, HH * (hh + 1))
        in_maps.append(
            {
                "xtq": np.ascontiguousarray(query[b].T).astype(ml_dtypes.bfloat16),
                "xtk": np.ascontiguousarray(key[b].T).astype(ml_dtypes.bfloat16),
                "xtv": np.ascontiguousarray(value[b].T).astype(ml_dtypes.bfloat16),
                "wqt": np.ascontiguousarray(Wq[cols, :].T).astype(ml_dtypes.bfloat16),
                "wkt": np.ascontiguousarray(Wk[cols, :].T).astype(ml_dtypes.bfloat16),
                "wvt": np.ascontiguousarray(Wv[cols, :].T).astype(ml_dtypes.bfloat16),
                "wot": np.ascontiguousarray(Wo[:, cols].T).astype(ml_dtypes.bfloat16),
                "bq": bq[cols],
                "bk": bk[cols],
                "bv": bv[cols],
            }
        )

    _cache["in_maps"] = in_maps
    res = run_bass_kernel_spmd(nc, in_maps, core_ids=list(range(NCORES)))
    out = np.empty((B, N, E), dtype=np.float32)
    for b in range(B):
        p = res.results[2 * b]["po"] + res.results[2 * b + 1]["po"]
        out[b] = p.T + bo
    return out


# revision 35
# speedup vs baseline: 1.0502x; 1.0502x over previous
"""MultiHeadAttention (B=4, N=2048, E=1024, H=16) on 8 TRN2 NeuronCores.

Sharding: core c handles batch b = c//2 and head-half hh = c%2 (8 heads,
512 embed dims). Each core computes Q/K/V projections for its 8 heads,
attention, and a partial output projection (contraction over its 512 c-dims).
Host sums the two partials per batch and adds the output bias.

Design: the kernel is scheduled to be ACT(exp)-bound. Per core the exp
work is 32 x [2048 x 512] = 33.5M elements = ~293us on the scalar engine
(1 elem/lane/cycle @ 1.2 GHz + 352-cycle per-ACTIVATE overhead); every
other engine is kept under that pace:

 - Attention runs in 16 head-PAIR slots (qb, t): heads (2t, 2t+1) live at
   partitions 0:64 / 64:128 of QT[t]/KT[t], so the two per-chunk score
   matmuls (contraction d=64) land on disjoint PE row groups
   (tile_position (0,0) / (64,0)) and execute CONCURRENTLY -- halving
   score time vs. serial d=64 matmuls.
 - Per chunk (1 key tile x 2 heads): 2 concurrent score MMs -> st PSUM
   [128,2,512], one exp ACTIVATE [128, 1024] -> at bf16, 2 attn@V MMs
   (V stationary [128,65] with a ones column producing the softmax
   denominator in row 64).
 - Scores are issued 2 chunks ahead of exp (st double-buffered); attnv
   lags exp by 2 chunks; the PE never sits between ACT and itself on the
   critical path, so ACT runs back-to-back.
 - Softmax normalization is moved OFF the critical path: ot PSUM banks
   are drained to SBUF immediately (freeing them for the next pair =
   single-buffered ot), denominators of both heads are DMA'd into rows
   0/32 of one tile so ONE DVE reciprocal (iterative, ~6.5 cyc/elem)
   serves the pair, the reciprocal row is broadcast to 64 partitions by
   two col-tiled K=1 matmuls into one PSUM bank, and DVE multiplies
   produce otn [128, 512] bf16 (head B's value half is partition-shifted
   64->128 via an SBUF->SBUF DMA since DVE lanes cannot cross partitions).
 - Projections: K fully + Q(qb0,t0) upfront (scores start ~15us in);
   V projection, remaining Q projection, output projection and the
   reciprocal-broadcast matmuls are issued as PE fillers inside the
   ACT-bound slots.
 - PSUM budget (8 banks): st 2x[128,2,512] (4) + ot_A + ot_B (2) +
   shared proj/bcast/outproj pool (2).
"""
import sys

sys.path.insert(0, "/opt/trn_rl_repo")

import numpy as np
import ml_dtypes

B, N, E = 4, 2048, 1024
NCORES = 8
HH = 512          # embed dims (8 heads x 64) per core
D = 64
NHEAD = 8         # heads per core
NKT = 16          # 128-wide key tiles
NPAIR = 16        # head-pair slots: pair p = (qb, t) = (p//4, p%4)

_cache = {}


def _build():
    import concourse.mybir as mybir
    import concourse.tile as tile
    import concourse.bacc as bacc

    F32 = mybir.dt.float32
    BF16 = mybir.dt.bfloat16
    EXP = mybir.ActivationFunctionType.Exp

    nc = bacc.Bacc(trn_type="TRN2")

    xtq = nc.dram_tensor("xtq", [E, N], BF16, kind="ExternalInput")
    xtk = nc.dram_tensor("xtk", [E, N], BF16, kind="ExternalInput")
    xtv = nc.dram_tensor("xtv", [E, N], BF16, kind="ExternalInput")
    wqt = nc.dram_tensor("wqt", [E, HH], BF16, kind="ExternalInput")
    wkt = nc.dram_tensor("wkt", [E, HH], BF16, kind="ExternalInput")
    wvt = nc.dram_tensor("wvt", [E, HH], BF16, kind="ExternalInput")
    wot = nc.dram_tensor("wot", [HH, E], BF16, kind="ExternalInput")
    bq = nc.dram_tensor("bq", [HH], F32, kind="ExternalInput")
    bk = nc.dram_tensor("bk", [HH], F32, kind="ExternalInput")
    bv = nc.dram_tensor("bv", [HH], F32, kind="ExternalInput")
    po = nc.dram_tensor("po", [E, N], F32, kind="ExternalOutput")

    with tile.TileContext(nc) as tc:
        with (
            tc.tile_pool(name="consts", bufs=1) as consts,
            tc.tile_pool(name="qk", bufs=1) as qk_pool,
            tc.tile_pool(name="vx", bufs=1) as v_pool,
            tc.tile_pool(name="wop", bufs=1) as wo_pool,
            tc.tile_pool(name="wp", bufs=1) as w_pool,
            tc.tile_pool(name="xt", bufs=5) as xt_pool,
            tc.tile_pool(name="at", bufs=17) as at_pool,
            tc.tile_pool(name="osb", bufs=2) as osb_pool,
            tc.tile_pool(name="dn", bufs=2) as dn_pool,
            tc.tile_pool(name="otn", bufs=2) as otn_pool,
            tc.tile_pool(name="ob", bufs=2) as ob_pool,
            tc.tile_pool(name="ojs", bufs=2) as ojs_pool,
            tc.tile_pool(name="st", bufs=2, space="PSUM") as st_ps,
            tc.tile_pool(name="ot", bufs=1, space="PSUM") as ot_ps,
            tc.tile_pool(name="pb", bufs=2, space="PSUM") as pb_ps,
        ):
            # ---------------- constants ----------------
            ones_b = consts.tile([128, 128], BF16)
            nc.vector.memset(ones_b, 1.0)
            onescol_f = consts.tile([128, NHEAD, 1], F32)
            nc.vector.memset(onescol_f, 1.0)
            # preload the ACT exp table during the ramp (~2.7us
            # ACT_TABLE_LOAD otherwise lands on the first attention exp)
            exp_warm = consts.tile([1, 8], F32)
            nc.vector.memset(exp_warm, 0.0)
            exp_warm_o = consts.tile([1, 8], F32)
            nc.scalar.activation(exp_warm_o, exp_warm, EXP)

            bq_t = consts.tile([128, 4], F32)
            bk_t = consts.tile([128, 4], F32)
            nc.sync.dma_start(out=bq_t, in_=bq.ap().rearrange("(t p) -> p t", p=128))
            nc.sync.dma_start(out=bk_t, in_=bk.ap().rearrange("(t p) -> p t", p=128))
            bv_row = consts.tile([1, HH], F32)
            nc.sync.dma_start(out=bv_row, in_=bv.ap().rearrange("(a n) -> a n", a=1))
            bv_row_b = consts.tile([1, HH], BF16)
            nc.vector.tensor_copy(bv_row_b, bv_row)
            bv_bc = consts.tile([128, HH], F32)

            # persistent activations
            QT = [qk_pool.tile([128, N], BF16, tag=f"qt{t}", name=f"qt{t}") for t in range(4)]
            KT = [qk_pool.tile([128, N], BF16, tag=f"kt{t}", name=f"kt{t}") for t in range(4)]
            VE = [v_pool.tile([128, NHEAD, D + 1], BF16, tag=f"ve{g}", name=f"ve{g}") for g in range(16)]
            wo_t = wo_pool.tile([128, 4, E], BF16, tag="wo")

            # ---------------- weight / x loads ----------------
            wts = {}

            def load_w(nm, wdram, engine=None):
                wt = w_pool.tile([128, 8, HH], BF16, tag=f"w{nm}", name=f"wt_{nm}")
                (engine or nc.sync).dma_start(
                    out=wt, in_=wdram.ap().rearrange("(kt p) n -> p kt n", p=128)
                )
                wts[nm] = wt

            xts = {}

            def load_x(which, xdram, th, engine=None):
                # 'q' tiles live across several slots (QP(qb, dt) spread over
                # 3 slots) -- separate 2-buf tag so k/v rotation can't evict
                # them before their late readers are emitted
                tag, bufs = ("xq", 2) if which == "q" else ("xt", None)
                xt = xt_pool.tile(
                    [128, 8, 512], BF16, tag=tag, bufs=bufs, name=f"x{which}{th}"
                )
                (engine or nc.sync).dma_start(
                    out=xt,
                    in_=xdram.ap().rearrange("(kt p) n -> p kt n", p=128)[
                        :, :, 512 * th : 512 * (th + 1)
                    ],
                )
                xts[(which, th)] = xt

            def qk_group(which, th, dt):
                xt = xts[(which, th)]
                wt = wts[which]
                dest = (QT if which == "q" else KT)[dt]
                bias = bq_t if which == "q" else bk_t
                ps = pb_ps.tile([128, 512], F32, tag="pb", name=f"pp{which}{th}{dt}")
                for kt in range(8):
                    nc.tensor.matmul(
                        ps,
                        wt[:, kt, 128 * dt : 128 * (dt + 1)],
                        xt[:, kt, :],
                        start=(kt == 0),
                        stop=(kt == 7),
                    )
                nc.vector.tensor_scalar_add(
                    dest[:, 512 * th : 512 * (th + 1)], ps, bias[:, dt : dt + 1]
                )

            vp_emitted = [False] * 16

            def v_group(th, tt):
                xt = xts[("v", th)]
                g = 4 * th + tt
                ps = pb_ps.tile([128, 512], F32, tag="pb", name=f"pv{g}")
                for kt in range(8):
                    nc.tensor.matmul(
                        ps,
                        xt[:, kt, 128 * tt : 128 * (tt + 1)],
                        wts["v"][:, kt, :],
                        start=(kt == 0),
                        stop=(kt == 7),
                    )
                nc.vector.tensor_add(
                    VE[g][:, :, 0:D],
                    ps.rearrange("p (h d) -> p h d", h=NHEAD),
                    bv_bc.rearrange("p (h d) -> p h d", h=NHEAD),
                )
                nc.vector.tensor_copy(VE[g][:, :, D : D + 1], onescol_f)
                vp_emitted[g] = True

            def load_wo():
                nc.sync.dma_start(
                    out=wo_t, in_=wot.ap().rearrange("(ct p) n -> p ct n", p=128)
                )

            # ---------------- attention primitives ----------------
            at_tiles = {}
            ot_tiles = {}
            osb_tiles = {}
            r2_tiles = {}
            otn_tiles = {}
            bc_queue = []
            exp_count = [0]   # chunks exp'd (global)
            av_count = [0]    # chunks attnv'd (global)

            def sc(p, c):
                qb, t = divmod(p, 4)
                q0 = 512 * qb
                st = st_ps.tile([128, 2, 512], F32, tag="st", name=f"st{p}_{c}")
                for i, par in enumerate((0, 64)):
                    nc.tensor.matmul(
                        st[:, i, :],
                        KT[t][par : par + 64, 128 * c : 128 * (c + 1)],
                        QT[t][par : par + 64, q0 : q0 + 512],
                        start=True,
                        stop=True,
                    )
                at = at_pool.tile([128, 2, 512], BF16, tag="at", name=f"at{p}_{c}")
                nc.scalar.activation(at, st, EXP, scale=0.125)
                at_tiles[(p, c)] = at
                exp_count[0] += 1

            def attnv_one(p, c):
                qb, t = divmod(p, 4)
                if c == 0:
                    ot_tiles[(p, 0)] = ot_ps.tile(
                        [128, 512], F32, tag="ota", name=f"ota{p}"
                    )
                    ot_tiles[(p, 1)] = ot_ps.tile(
                        [128, 512], F32, tag="otb", name=f"otb{p}"
                    )
                at = at_tiles.pop((p, c))
                for i in (0, 1):
                    h = 2 * t + i
                    nc.tensor.matmul(
                        ot_tiles[(p, i)][0:65, :],
                        VE[c][:, h, :],
                        at[:, i, :],
                        start=(c == 0),
                        stop=(c == 15),
                    )
                av_count[0] += 1
                if c == 15:
                    norm_copies(p)

            def norm_copies(p):
                # denominator rows first (tiny copies) so the den DMAs and
                # the reciprocal start ~1.5us earlier -- keeps the bc
                # broadcast matmul at c==6 of the next slot from stalling PE
                ota, otb = ot_tiles.pop((p, 0)), ot_tiles.pop((p, 1))
                rowa = dn_pool.tile([1, 512], F32, tag="rowa", bufs=1, name=f"rwa{p}")
                rowb = dn_pool.tile([1, 512], F32, tag="rowb", bufs=1, name=f"rwb{p}")
                nc.vector.tensor_copy(rowa, ota[64:65, :])
                nc.vector.tensor_copy(rowb, otb[64:65, :])
                den = dn_pool.tile([33, 512], F32, tag="den", bufs=1, name=f"den{p}")
                nc.sync.dma_start(out=den[0:1, :], in_=rowa)
                nc.sync.dma_start(out=den[32:33, :], in_=rowb)
                # drain ot value rows to SBUF (frees PSUM for the next pair)
                oa = osb_pool.tile([64, 512], F32, tag="osba", name=f"osba{p}")
                ob_ = osb_pool.tile([64, 512], F32, tag="osbb", name=f"osbb{p}")
                nc.vector.tensor_copy(oa, ota[0:64, :])
                nc.vector.tensor_copy(ob_, otb[0:64, :])
                r2 = dn_pool.tile([33, 512], BF16, tag="r2", bufs=1, name=f"r2{p}")
                with nc.allow_low_precision(reason="bf16 softmax recip"):
                    nc.vector.reciprocal(r2, den)
                # head B's value half must live at partitions 64:128 for the
                # out-projection moving operand; DVE lanes can't cross
                # partitions, so shift via SBUF->SBUF DMA
                obig = ob_pool.tile([128, 512], F32, tag="ob", name=f"ob{p}")
                nc.sync.dma_start(out=obig[64:128, :], in_=ob_)
                osb_tiles[p] = (oa, obig)
                r2_tiles[p] = r2
                bc_queue.append(p)

            oj_queue = []

            def bc_mul(p):
                qb, t = divmod(p, 4)
                r2 = r2_tiles.pop(p)
                bc = pb_ps.tile([128, 512], F32, tag="pb", name=f"bc{p}")
                nc.tensor.matmul(
                    bc[0:64, :], ones_b[0:1, 0:64], r2[0:1, :], start=True, stop=True
                )
                nc.tensor.matmul(
                    bc[64:128, :], ones_b[32:33, 0:64], r2[32:33, :],
                    start=True, stop=True,
                )
                oa, obig = osb_tiles.pop(p)
                otn = otn_pool.tile([128, 512], BF16, tag=f"otn{t}", name=f"otn{p}")
                nc.vector.tensor_mul(otn[0:64, :], oa[0:64, :], bc[0:64, :])
                nc.vector.tensor_mul(otn[64:128, :], obig[64:128, :], bc[64:128, :])
                otn_tiles[(qb, t)] = otn
                if t == 3:
                    oj_queue.extend((qb, jt) for jt in range(8))

            def oj_group(qb, jt, pool_tag=None):
                pool, tag = pool_tag or (pb_ps, "pb")
                pj = pool.tile([128, 512], F32, tag=tag, name=f"oj{qb}_{jt}")
                for ct in range(4):
                    nc.tensor.matmul(
                        pj,
                        wo_t[:, ct, 128 * jt : 128 * (jt + 1)],
                        otn_tiles[(qb, ct)],
                        start=(ct == 0),
                        stop=(ct == 3),
                    )
                ojsb = ojs_pool.tile([128, 512], F32, tag="ojs", name=f"ojsb{qb}_{jt}")
                nc.vector.tensor_copy(ojsb, pj)
                nc.sync.dma_start(
                    out=po.ap()[128 * jt : 128 * (jt + 1), 512 * qb : 512 * (qb + 1)],
                    in_=ojsb,
                )

            # attnv drain: lag >= min_lag chunks behind exp (keeps the PE
            # from stalling on ACT mid-slot); gated on VE emission
            def drain_attnv(max_items, min_lag=2):
                n = 0
                while n < max_items and av_count[0] <= exp_count[0] - min_lag:
                    p_a, c_a = divmod(av_count[0], 16)
                    if not vp_emitted[c_a]:
                        break
                    attnv_one(p_a, c_a)
                    n += 1

            # ---------------- ramp DMAs ----------------
            # xtv loads are emitted just-in-time inside the slot-0 script so
            # their xt-pool buffer reuse lands after the K-proj readers of
            # the evicted xtk tiles are emitted (pool WAR only orders
            # against already-emitted readers).
            # spread across the three DMA-capable queues: K path on sync,
            # Q path on the scalar queue (ACT is idle until the first exp),
            # V path + wo on the gpsimd software DGE
            load_w("k", wkt)
            load_x("k", xtk, 0)
            load_w("q", wqt)
            load_x("q", xtq, 0)
            for th in (1, 2, 3):
                load_x("k", xtk, th)
            load_w("v", wvt)

            # bv broadcast to all partitions via K=1 matmul
            bc0 = pb_ps.tile([128, HH], F32, tag="pb", name="bvbc")
            nc.tensor.matmul(bc0, ones_b[0:1, :], bv_row_b, start=True, stop=True)
            nc.vector.tensor_copy(bv_bc, bc0)

            # ---------------- slot 0 (pair 0): ramp script ----------------
            # Pair 0 = (qb0, t0): chunk c needs only KP(th=c//4, dt=0), so
            # KP(th, 0) goes on the critical path and the dt>0 K-proj groups
            # (KT[1..3], needed from slot 1 on) interleave behind it.
            qk_group("k", 0, 0)
            qk_group("q", 0, 0)
            load_x("v", xtv, 0)   # 5th xt buf (fresh)
            sc(0, 0)
            sc(0, 1)
            qk_group("k", 1, 0)
            qk_group("k", 0, 1)
            sc(0, 2)
            sc(0, 3)
            qk_group("k", 2, 0)
            qk_group("k", 0, 2)
            sc(0, 4)
            sc(0, 5)
            qk_group("k", 3, 0)
            qk_group("k", 0, 3)
            load_x("v", xtv, 1)   # reuses xtk0 buf (KP(0,*) emitted)
            sc(0, 6)
            sc(0, 7)
            qk_group("k", 1, 1)
            qk_group("k", 2, 1)
            sc(0, 8)
            sc(0, 9)
            qk_group("k", 3, 1)
            qk_group("k", 1, 2)
            sc(0, 10)
            sc(0, 11)
            qk_group("k", 2, 2)
            qk_group("k", 3, 2)
            sc(0, 12)
            sc(0, 13)
            qk_group("k", 1, 3)
            load_x("v", xtv, 2)   # reuses xtk1 buf (KP(1,*) emitted)
            sc(0, 14)
            sc(0, 15)
            qk_group("k", 2, 3)
            qk_group("k", 3, 3)
            load_x("v", xtv, 3)   # reuses xtk2 buf (KP(2,*) emitted)
            qk_group("q", 0, 1)
            # V-proj for the first 10 key tiles while ACT drains the pair-0
            # exp backlog (PE is otherwise idle at slot-0 end)
            v_group(0, 0)
            v_group(0, 1)
            drain_attnv(2)
            v_group(0, 2)
            v_group(0, 3)
            drain_attnv(2)
            v_group(1, 0)
            v_group(1, 1)
            drain_attnv(2)
            v_group(1, 2)
            v_group(1, 3)
            drain_attnv(2)
            v_group(2, 0)
            v_group(2, 1)
            load_wo()
            load_x("q", xtq, 1)
            drain_attnv(4)

            # ---------------- slots 1..15 ----------------
            # filler queues per slot (projection groups during the ramp,
            # Q-projection blocks just-in-time for later q-blocks).
            # All V-proj groups go in slot 1 so attnv (VE-gated) never
            # lags exp by more than the at-pool depth.
            fillers = {
                1: [lambda: v_group(2, 2), lambda: v_group(2, 3),
                    lambda: v_group(3, 0), lambda: v_group(3, 1),
                    lambda: v_group(3, 2), lambda: v_group(3, 3),
                    lambda: qk_group("q", 0, 2)],
                2: [lambda: qk_group("q", 0, 3)],
                3: [lambda: qk_group("q", 1, 0)],
                4: [lambda: qk_group("q", 1, 1)],
                5: [lambda: qk_group("q", 1, 2), lambda: load_x("q", xtq, 2)],
                6: [lambda: qk_group("q", 1, 3)],
                7: [lambda: qk_group("q", 2, 0)],
                8: [lambda: qk_group("q", 2, 1)],
                9: [lambda: qk_group("q", 2, 2), lambda: load_x("q", xtq, 3)],
                10: [lambda: qk_group("q", 2, 3)],
                11: [lambda: qk_group("q", 3, 0)],
                12: [lambda: qk_group("q", 3, 1)],
                13: [lambda: qk_group("q", 3, 2)],
                14: [lambda: qk_group("q", 3, 3)],
            }
            for s in range(1, NPAIR):
                fq = list(fillers.get(s, []))
                filler_slots = set(range(16)) if s == 1 else {2, 5}
                for c in range(16):
                    sc(s, c)
                    if fq and c in filler_slots:
                        fq.pop(0)()
                    if c in (8, 11) and bc_queue:
                        bc_mul(bc_queue.pop(0))
                    if c in (10, 13) and oj_queue:
                        oj_group(*oj_queue.pop(0))
                    drain_attnv(3)
                while fq:
                    fq.pop(0)()
                drain_attnv(4)

            # ---------------- tail ----------------
            while av_count[0] < NPAIR * 16:
                drain_attnv(16, min_lag=1)
            while bc_queue:
                bc_mul(bc_queue.pop(0))
            # tail out-projections 4-wide: the attnv accumulator banks are
            # dead after pair 15's drain, reuse them as extra oj slots
            tail_pools = [(pb_ps, "pb"), (ot_ps, "ota"), (pb_ps, "pb"), (ot_ps, "otb")]
            i = 0
            while oj_queue:
                oj_group(*oj_queue.pop(0), pool_tag=tail_pools[i % 4])
                i += 1

    nc.compile()
    return nc


def _get_nc():
    if "nc" not in _cache:
        _cache["nc"] = _build()
    return _cache["nc"]


def kernel(query, key, value, Wq, bq, Wk, bk, Wv, bv, Wo, bo):
    from concourse.bass_utils import run_bass_kernel_spmd

    nc = _get_nc()

    query = np.asarray(query, dtype=np.float32)
    key = np.asarray(key, dtype=np.float32)
    value = np.asarray(value, dtype=np.float32)
    Wq, Wk, Wv, Wo = (np.asarray(w, dtype=np.float32) for w in (Wq, Wk, Wv, Wo))
    bq, bk, bv, bo = (np.asarray(b, dtype=np.float32) for b in (bq, bk, bv, bo))

    in_maps = []
    for c in range(NCORES):
        b, hh = c // 2, c % 2
        cols = slice(HH * hh, HH * (hh + 1))
        in_maps.append(
            {
                "xtq": np.ascontiguousarray(query[b].T).astype(ml_dtypes.bfloat16),
                "xtk": np.ascontiguousarray(key[b].T).astype(ml_dtypes.bfloat16),
                "xtv": np.ascontiguousarray(value[b].T).astype(ml_dtypes.bfloat16),
                "wqt": np.ascontiguousarray(Wq[cols, :].T).astype(ml_dtypes.bfloat16),
                "wkt": np.ascontiguousarray(Wk[cols, :].T).astype(ml_dtypes.bfloat16),
                "wvt": np.ascontiguousarray(Wv[cols, :].T).astype(ml_dtypes.bfloat16),
                "wot": np.ascontiguousarray(Wo[:, cols].T).astype(ml_dtypes.bfloat16),
                "bq": bq[cols],
                "bk": bk[cols],
                "bv": bv[cols],
            }
        )

    _cache["in_maps"] = in_maps
    res = run_bass_kernel_spmd(nc, in_maps, core_ids=list(range(NCORES)))
    out = np.empty((B, N, E), dtype=np.float32)
    for b in range(B):
        p = res.results[2 * b]["po"] + res.results[2 * b + 1]["po"]
        out[b] = p.T + bo
    return out
